# revision 36
# baseline (speedup 1.0000x reference)
"""
AngularPenaltySMLoss on 8 Trainium2 NeuronCores, pure data parallel.

Math (reference):
    r = ||x_i||;  soft = relu(1.5 - r) + relu(r - 2)
    xn = x / max(r, eps);  wf = xn @ W.T   (W is [10, 2])
    t = wf[i, label_i];  num = S*cos(arccos(clip(t)) + M)
    den = exp(num) + sum_c exp(S*wf_c) - exp(S*t)
    loss = -mean(num - log(den)) + LBDA*mean(soft)/2

Kernel strategy (per core, 524288 rows as [128 x 4096], two passes of
F=2048, all VectorE elementwise in bf16):
  - Host pre-gathers the target-class weights wl = W_bf16[label] and ships
    them as two bf16 planes: the target dot tt = y0*wl0 + y1*wl1 is then
    BIT-IDENTICAL to the class dot z_label (same inputs, same ops), so
    excl_sum = sum_c exp(S z_c) - exp(S tt) cancels exactly -- no
    per-class label masks needed at all.
  - W has w[c+5] == -w[c] exactly (checked on host): only 5 class dots are
    computed; exp(+S z) and exp(-S z) come from two ScalarE activations on
    the same tile (scale=+-S). Falls back to a 10-dot graph otherwise.
  - sum_c exp(.) - exp(S tt) accumulates on TensorE via identity-matmul
    chains into PSUM (idm = -I for the target term).
  - Whole tail folds into one accumulated Ln:
      -L = ln(1 + excl * e^{-num}) summed by the activation's accum_out.
    num is never materialized or summed; exp(-num) comes from one Exp.
  - softloss partial sums ride free on tensor_scalar accum_out:
      a = min(r,1.5)-1.5 (= -relu(1.5-r)),  b = max(r,2)-2 (= relu(r-2)).
  - Host sums the 8 cores' [128, 6] partial tiles.
"""

import math
import os
import sys

import numpy as np

for _p in ("/opt/trn_rl_repo", "/root/.axon_site/_ro/trn_rl_repo"):
    if os.path.isdir(_p) and _p not in sys.path:
        sys.path.insert(0, _p)

from contextlib import ExitStack

from concourse import bacc, bass, tile
from concourse import mybir
from concourse.bass_utils import run_bass_kernel_spmd

# ---- problem constants (hardcoded; kernel.py must be self-contained) ----
S = 30.0
M = 0.5
LBDA = 1.0
N = 4_194_304
N_CORES = 8
P = 128
NC_ROWS = N // N_CORES            # 524288 rows per core
PF = NC_ROWS // P                 # 4096 per partition
F = 2048                          # free-dim per pass
NPASS = PF // F                   # 2
NCLS = 10
MM_N = 512                        # one PSUM bank of fp32 per matmul

COS_M = math.cos(M)
TAN_M = math.tan(M)
LN_TAN_M = math.log(TAN_M)

f32 = mybir.dt.float32
f32r = mybir.dt.float32r
bf16 = mybir.dt.bfloat16
Alu = mybir.AluOpType
Act = mybir.ActivationFunctionType

# ln(1 + excl*e^-num) can reach e^58, past ScalarE Ln's 2^64 domain: shift
# everything by e^-LSH (folded into e_nn's bias and the Ln's bias; host
# adds LSH back per row).
LSH = 30.0
# activation-bias constants, shipped as a tiny DMA'd input (col i of "kb")
# so no gpsimd memset + all-engine barrier is needed at startup
_CONST_BIASES = (1e-30, 1e-12, LN_TAN_M, math.exp(-LSH), -LSH, 0.0)


def _patch_act_tables():
    """Force Exp and Ln onto the one table set containing both
    (natural_log_exp_and_others) so no ~2.7us table reloads occur at
    ln<->exp boundaries."""
    import concourse.hw_specs as hw_specs
    import concourse.bacc as bacc_mod

    orig = hw_specs.get_activation_tables
    if getattr(bacc_mod.get_activation_tables, "_k_patched", False):
        return
    ours = {Act.Exp, Act.Ln, Act.Copy, Act.Identity}

    def patched(module_arch):
        tables = orig(module_arch)
        target = "natural_log_exp_and_others"
        assert target in tables and ours <= tables[target], (
            target, tables.get(target))
        for name in tables:
            if name != target:
                tables[name] = tables[name] - ours
        return tables

    patched._k_patched = True
    bacc_mod.get_activation_tables = patched


def _build_graph(sym: bool):
    _patch_act_tables()
    nc = bacc.Bacc(
        "TRN2", target_bir_lowering=False, debug=False, enable_asserts=False
    )
    npairs = 5 if sym else NCLS
    kb_d = nc.dram_tensor("kb", [P, 8], f32, kind="ExternalInput").ap()
    x0_d = nc.dram_tensor("x0", [P, PF], bf16, kind="ExternalInput").ap()
    x1_d = nc.dram_tensor("x1", [P, PF], bf16, kind="ExternalInput").ap()
    wl0_d = nc.dram_tensor("wl0", [P, PF], bf16, kind="ExternalInput").ap()
    wl1_d = nc.dram_tensor("wl1", [P, PF], bf16, kind="ExternalInput").ap()
    wq_d = nc.dram_tensor("wq", [P, 2 * npairs], f32, kind="ExternalInput").ap()
    idp_d = nc.dram_tensor("idp", [P, P], f32, kind="ExternalInput").ap()
    idm_d = nc.dram_tensor("idm", [P, P], f32, kind="ExternalInput").ap()
    out_d = nc.dram_tensor("out", [P, 7], f32, kind="ExternalOutput").ap()

    with tile.TileContext(nc) as tc, ExitStack() as ctx:
        _emit(ctx, tc, nc, sym, kb_d, x0_d, x1_d, wl0_d, wl1_d, wq_d, idp_d,
              idm_d, out_d)
    nc.compile()
    return nc


def _emit(ctx, tc, nc, sym, kb_d, x0_d, x1_d, wl0_d, wl1_d, wq_d, idp_d, idm_d,
          out_d):
    npairs = 5 if sym else NCLS
    # bufs=2 pools hold tiles whose next-pass write is emitted before this
    # pass's last read (pipelined emission would WAR-deadlock at bufs=1)
    const = ctx.enter_context(tc.tile_pool(name="const", bufs=1))
    # bufs=1 on inputs gates pass-1 loads behind pass-0's last reads, so the
    # DMA engines aren't splitting bandwidth 8 ways while pass 0 waits for
    # its first bytes
    dma_p = ctx.enter_context(tc.tile_pool(name="dma", bufs=1))
    h1 = ctx.enter_context(tc.tile_pool(name="h1", bufs=1))
    h2 = ctx.enter_context(tc.tile_pool(name="h2", bufs=2))
    h2b = ctx.enter_context(tc.tile_pool(name="h2b", bufs=1))
    zab = ctx.enter_context(tc.tile_pool(name="zab", bufs=1))
    zcp = ctx.enter_context(tc.tile_pool(name="zcp", bufs=2))
    ep = ctx.enter_context(tc.tile_pool(name="ep", bufs=3))
    etp = ctx.enter_context(tc.tile_pool(name="etp", bufs=2))
    tlA = ctx.enter_context(tc.tile_pool(name="tlA", bufs=2))
    tl = ctx.enter_context(tc.tile_pool(name="tl", bufs=1))
    psum = ctx.enter_context(tc.tile_pool(name="psum", bufs=2, space="PSUM"))

    # one-time constants; bias constants are columns of kb, registered in
    # const_aps as tile-slice APs so activation reads are dep-tracked on
    # the DMA (no barrier needed)
    kb = const.tile([P, 8], f32, tag="kb")
    nc.sync.dma_start(kb[:], kb_d[:])
    for i, v in enumerate(_CONST_BIASES):
        nc.const_aps.aps[(f32, v)] = kb[:, i : i + 1]
    wq = const.tile([P, 2 * npairs], f32, tag="wq")
    nc.sync.dma_start(wq[:], wq_d[:])
    idp = const.tile([P, P], f32r, tag="idp")
    nc.sync.dma_start(idp[:], idp_d[:].bitcast(f32r))
    idm = const.tile([P, P], f32r, tag="idm")
    nc.sync.dma_start(idm[:], idm_d[:].bitcast(f32r))
    sacc = const.tile([P, 7], f32, tag="sacc")

    state = {}

    def h12(t, nchunk=1):
        """DMA + r^2 + 1/r, optionally sub-chunked to shorten the serial
        V->S->V warm-up chain of the very first pass."""
        x0t = dma_p.tile([P, F], bf16, tag="x0t")
        x1t = dma_p.tile([P, F], bf16, tag="x1t")
        wl0t = dma_p.tile([P, F], bf16, tag="wl0t")
        wl1t = dma_p.tile([P, F], bf16, tag="wl1t")
        fd = F // nchunk
        for k in range(nchunk):
            ck = bass.ts(t * nchunk + k, fd)
            cl = bass.ts(k, fd)
            nc.sync.dma_start(x0t[:, cl], x0_d[:, ck])
            nc.sync.dma_start(x1t[:, cl], x1_d[:, ck])
        sl = bass.ts(t, F)
        # wl loads issued from the Scalar queue mid-head: the issue point
        # naturally staggers them behind the x loads so the very first x
        # chunk isn't sharing DMA bandwidth with everything else
        wl_eng = nc.scalar if t == 0 else nc.sync

        sq0 = h1.tile([P, F], bf16, tag="sq0")
        sq1 = h1.tile([P, F], bf16, tag="sq1")
        rsq = h2.tile([P, F], bf16, tag="rsq")
        lr = h1.tile([P, F], f32, tag="lr")
        invr = h2.tile([P, F], bf16, tag="invr")
        fc = F // nchunk
        for k in range(nchunk):
            ck = bass.ts(k, fc)
            nc.vector.tensor_mul(sq0[:, ck], x0t[:, ck], x0t[:, ck])
            nc.vector.tensor_mul(sq1[:, ck], x1t[:, ck], x1t[:, ck])
            nc.vector.tensor_add(rsq[:, ck], sq0[:, ck], sq1[:, ck])
            nc.scalar.activation(lr[:, ck], rsq[:, ck], Act.Ln, bias=1e-30)
            nc.scalar.activation(invr[:, ck], lr[:, ck], Act.Exp, scale=-0.5)
            if k == 0:
                wl_eng.dma_start(wl0t[:], wl0_d[:, sl])
                wl_eng.dma_start(wl1t[:], wl1_d[:, sl])
        state[t] = dict(x0t=x0t, x1t=x1t, wl0t=wl0t, wl1t=wl1t, rsq=rsq,
                        invr=invr)

    def h3(t):
        """Unit vector, target dot, exp(S*t) + start of the PSUM chain,
        then r/soft accums (kept after tt so ScalarE unblocks early)."""
        st = state[t]
        x0t, x1t, invr, rsq = st["x0t"], st["x1t"], st["invr"], st["rsq"]
        y0 = h2b.tile([P, F], bf16, tag="y0")
        nc.vector.tensor_mul(y0[:], x0t[:], invr[:])
        y1 = h2b.tile([P, F], bf16, tag="y1")
        nc.vector.tensor_mul(y1[:], x1t[:], invr[:])
        t1 = h1.tile([P, F], bf16, tag="t1")
        nc.vector.tensor_mul(t1[:], y0[:], st["wl0t"][:])
        t2 = h1.tile([P, F], bf16, tag="t2")
        nc.vector.tensor_mul(t2[:], y1[:], st["wl1t"][:])
        tt = h2b.tile([P, F], bf16, tag="tt")
        nc.vector.tensor_add(tt[:], t1[:], t2[:])

        e_t = etp.tile([P, F], f32r, tag="et")
        nc.scalar.activation(e_t[:], tt[:], Act.Exp, scale=S)
        es_ps = psum.tile([P, F], f32, tag="es")
        for k in range(F // MM_N):
            ck = bass.ts(k, MM_N)
            nc.tensor.matmul(es_ps[:, ck], idm[:], e_t[:, ck], start=True, stop=False)

        # r on the otherwise-idle GpSimd engine (feeds only the soft sums)
        rs = h1.tile([P, F], bf16, tag="rs")
        nc.gpsimd.tensor_mul(rs[:], rsq[:], invr[:])
        state[t].update(y0=y0, y1=y1, tt=tt, rs=rs, es=es_ps)

    def soft_accums(t):
        """Soft-loss partial sums; spliced late in the class loop so they
        don't block the dot/exp critical path. accum_out =
        reduce<op1=add>(op0 result): Sa = sum(min(r,1.5)),
        Sb = sum(max(r,2)); host: Sb - Sa - 0.5*N."""
        rs = state[t]["rs"]
        trash_a = h1.tile([P, F], bf16, tag="trash_a")
        nc.vector.tensor_scalar(
            trash_a[:], rs[:], 1.5, None, Alu.min, Alu.add,
            accum_out=sacc[:, 3 * t + 1 : 3 * t + 2],
        )
        trash_b = h1.tile([P, F], bf16, tag="trash_b")
        nc.vector.tensor_scalar(
            trash_b[:], rs[:], 2.0, None, Alu.max, Alu.add,
            accum_out=sacc[:, 3 * t + 2 : 3 * t + 3],
        )

    def tail_pre(t):
        """clip(t), 1-t^2 -- only needs tt; spliced into the class loop."""
        st = state[t]
        tcl = tlA.tile([P, F], bf16, tag="tcl")
        nc.vector.tensor_scalar(tcl[:], st["tt"][:], 1.0, -1.0, Alu.min, Alu.max)
        tsq = tl.tile([P, F], bf16, tag="tsq")
        nc.vector.tensor_mul(tsq[:], tcl[:], tcl[:])
        u = tlA.tile([P, F], bf16, tag="u")
        nc.vector.tensor_scalar(u[:], tsq[:], -1.0, 1.0, Alu.mult, Alu.add)
        state[t].update(tcl=tcl, u=u)

    def classes(t, spliced):
        """Class dot/exp/matmul loop for pass t; `spliced` maps a class
        index to a list of closures emitted after that class (the previous
        pass's tail, interleaved to hide V<->S ping-pong)."""
        st = state[t]
        y0, y1, es_ps = st["y0"], st["y1"], st["es"]
        n_exp = 2 if sym else 1
        for c in range(npairs):
            za = zab.tile([P, F], bf16, tag="za")
            nc.vector.tensor_scalar(
                za[:], y0[:], wq[:, 2 * c : 2 * c + 1], None, Alu.mult
            )
            zb = zab.tile([P, F], bf16, tag="zb")
            nc.vector.tensor_scalar(
                zb[:], y1[:], wq[:, 2 * c + 1 : 2 * c + 2], None, Alu.mult
            )
            zc = zcp.tile([P, F], bf16, tag="zc")
            nc.vector.tensor_add(zc[:], za[:], zb[:])
            for j in range(n_exp):
                e = ep.tile([P, F], f32r, tag="e")
                nc.scalar.activation(e[:], zc[:], Act.Exp, scale=(S, -S)[j])
                last = (c == npairs - 1) and (j == n_exp - 1)
                for k in range(F // MM_N):
                    ck = bass.ts(k, MM_N)
                    nc.tensor.matmul(
                        es_ps[:, ck], idp[:], e[:, ck], start=False, stop=last
                    )
            for fn in spliced.get(c, ()):
                fn()

    def tail_post_ops(t, parts):
        """Stage closures for pass t's tail over free-dim sub-ranges.
        parts: list of (offset, length, lden_col). Returns per-stage lists
        of closures (each stage covers all parts, so a later part's
        ScalarE leg hides an earlier part's VectorE leg and vice versa)."""
        st = state[t]
        # one allocation per tag per pass; parts write disjoint slices
        lnu = tl.tile([P, F], bf16, tag="lnu")
        sqru = tl.tile([P, F], bf16, tag="sqru")
        nump = tl.tile([P, F], bf16, tag="nump")
        e_nn = tl.tile([P, F], f32, tag="enn")
        w = tl.tile([P, F], f32, tag="w")
        trash = tl.tile([P, F], bf16, tag="trash_l")

        def mk(stage, p):
            off, ln, col = parts[p]
            fs = slice(off, off + ln)

            def s_lnu():
                nc.scalar.activation(lnu[:, fs], st["u"][:, fs], Act.Ln, bias=1e-12)
                nc.scalar.activation(
                    sqru[:, fs], lnu[:, fs], Act.Exp, scale=0.5, bias=LN_TAN_M
                )

            def v_nump():
                nc.vector.tensor_tensor(
                    nump[:, fs], st["tcl"][:, fs], sqru[:, fs], Alu.subtract
                )

            def s_enn():
                nc.scalar.activation(
                    e_nn[:, fs], nump[:, fs], Act.Exp, scale=-S * COS_M, bias=-LSH
                )

            def v_w():
                nc.vector.tensor_mul(w[:, fs], st["es"][:, fs], e_nn[:, fs])

            def s_lnacc():
                nc.scalar.activation(
                    trash[:, fs], w[:, fs], Act.Ln, bias=math.exp(-LSH),
                    accum_out=sacc[:, col : col + 1],
                )

            return [s_lnu, v_nump, s_enn, v_w, s_lnacc][stage]

        return [[mk(stage, p) for p in range(len(parts))] for stage in range(5)]

    # ---- software-pipelined emission ----
    # h12(1) must come after h3(0): with bufs=1 input tiles, pass-1 DMAs
    # wait on pass-0's last reads (y0/t1), which must precede them in the
    # engine queues
    h12(0, nchunk=2)
    h3(0)
    h12(1)
    classes(0, {0: [lambda: tail_pre(0)],
                2: [lambda: soft_accums(0)]})
    h3(1)
    spliced = {0: [lambda: tail_pre(1)],
               3: [lambda: soft_accums(1)]}
    stages = tail_post_ops(0, [(0, F, 0)])
    for i, fns in enumerate(stages):
        spliced.setdefault(min(1 + i, npairs - 1), []).extend(fns)
    classes(1, spliced)
    # final tail: two interleaved half-chunks to overlap the V<->S chain
    fstages = tail_post_ops(1, [(0, F // 2, 3), (F // 2, F // 2, 6)])
    for fns in fstages:
        for fn in fns:
            fn()

    nc.sync.dma_start(out_d[:], sacc[:])


_NC_CACHE = {}


def _get_graph(sym: bool):
    if sym not in _NC_CACHE:
        _NC_CACHE[sym] = _build_graph(sym)
    return _NC_CACHE[sym]


def kernel(x, labels, weight):
    import ml_dtypes

    BF = ml_dtypes.bfloat16
    x = np.asarray(x, dtype=np.float32)
    labels = np.asarray(labels).astype(np.int64)
    w = np.asarray(weight, dtype=np.float32)

    wb = w.astype(BF)
    sym = w.shape[0] == NCLS and np.array_equal(
        wb[NCLS // 2 :], -wb[: NCLS // 2]
    )
    npairs = 5 if sym else NCLS
    nc = _get_graph(sym)

    wq = np.ascontiguousarray(
        np.tile(wb[:npairs].astype(np.float32).reshape(1, 2 * npairs), (P, 1))
    )
    idp = np.eye(P, dtype=np.float32)
    idm = -idp
    kbv = np.zeros(8, dtype=np.float32)
    kbv[: len(_CONST_BIASES)] = _CONST_BIASES
    kb = np.tile(kbv.reshape(1, 8), (P, 1))

    wl = wb[labels]                      # [N, 2] bf16 gather on host
    x0 = x[:, 0].astype(BF)
    x1 = x[:, 1].astype(BF)

    in_maps = []
    for i in range(N_CORES):
        sl = slice(i * NC_ROWS, (i + 1) * NC_ROWS)
        in_maps.append(
            {
                "x0": np.ascontiguousarray(x0[sl]).reshape(P, PF),
                "x1": np.ascontiguousarray(x1[sl]).reshape(P, PF),
                "wl0": np.ascontiguousarray(wl[sl, 0]).reshape(P, PF),
                "wl1": np.ascontiguousarray(wl[sl, 1]).reshape(P, PF),
                "wq": wq,
                "idp": idp,
                "idm": idm,
                "kb": kb,
            }
        )

    trace = os.environ.get("KTRACE", "0") == "1"
    res = run_bass_kernel_spmd(nc, in_maps, core_ids=list(range(N_CORES)), trace=trace)
    if getattr(res, "exec_time_ns", None):
        print(f"HW exec time: {res.exec_time_ns} ns")

    lden_sum = 0.0
    soft_sum = -0.5 * N
    for i in range(N_CORES):
        o = np.asarray(res.results[i]["out"], dtype=np.float64)
        lden_sum += o[:, 0].sum() + o[:, 3].sum() + o[:, 6].sum()
        soft_sum += (o[:, 2].sum() - o[:, 1].sum()) + (o[:, 5].sum() - o[:, 4].sum())

    loss = lden_sum / N + LSH + (LBDA / 2.0) * (soft_sum / N)
    return np.float32(loss)


if __name__ == "__main__":
    rng = np.random.default_rng(0)
    x = rng.standard_normal((N, 2), dtype=np.float32)
    labels = rng.integers(0, 10, size=(N,)).astype(np.int64)
    w = np.array(
        [[1, 0], [0.809, 0.588], [0.309, 0.951], [-0.309, 0.951], [-0.809, 0.588],
         [-1, 0], [-0.809, -0.588], [-0.309, -0.951], [0.309, -0.951], [0.809, -0.588]],
        dtype=np.float32,
    )
    print(kernel(x, labels, w))


# revision 46
# speedup vs baseline: 1.0310x; 1.0310x over previous
"""
AngularPenaltySMLoss on 8 Trainium2 NeuronCores, pure data parallel.

Math (reference):
    r = ||x_i||;  soft = relu(1.5 - r) + relu(r - 2)
    xn = x / max(r, eps);  wf = xn @ W.T   (W is [10, 2])
    t = wf[i, label_i];  num = S*cos(arccos(clip(t)) + M)
    den = exp(num) + sum_c exp(S*wf_c) - exp(S*t)
    loss = -mean(num - log(den)) + LBDA*mean(soft)/2

Kernel strategy (per core, 524288 rows as [128 x 4096], two passes of
F=2048, all VectorE elementwise in bf16):
  - Host pre-gathers the target-class weights wl = W_bf16[label] and ships
    them as two bf16 planes: the target dot tt = y0*wl0 + y1*wl1 is then
    BIT-IDENTICAL to the class dot z_label (same inputs, same ops), so
    excl_sum = sum_c exp(S z_c) - exp(S tt) cancels exactly -- no
    per-class label masks needed at all.
  - W has w[c+5] == -w[c] exactly (checked on host): only 5 class dots are
    computed; exp(+S z) and exp(-S z) come from two ScalarE activations on
    the same tile (scale=+-S). Falls back to a 10-dot graph otherwise.
  - sum_c exp(.) - exp(S tt) accumulates on TensorE via identity-matmul
    chains into PSUM (idm = -I for the target term).
  - Whole tail folds into one accumulated Ln:
      -L = ln(1 + excl * e^{-num}) summed by the activation's accum_out.
    num is never materialized or summed; exp(-num) comes from one Exp.
  - softloss partial sums ride free on tensor_scalar accum_out:
      a = min(r,1.5)-1.5 (= -relu(1.5-r)),  b = max(r,2)-2 (= relu(r-2)).
  - Host sums the 8 cores' [128, 6] partial tiles.
"""

import math
import os
import sys

import numpy as np

for _p in ("/opt/trn_rl_repo", "/root/.axon_site/_ro/trn_rl_repo"):
    if os.path.isdir(_p) and _p not in sys.path:
        sys.path.insert(0, _p)

from contextlib import ExitStack

from concourse import bacc, bass, tile
from concourse import mybir
from concourse.bass_utils import run_bass_kernel_spmd

# ---- problem constants (hardcoded; kernel.py must be self-contained) ----
S = 30.0
M = 0.5
LBDA = 1.0
N = 4_194_304
N_CORES = 8
P = 128
NC_ROWS = N // N_CORES            # 524288 rows per core
PF = NC_ROWS // P                 # 4096 per partition
F = 2048                          # free-dim per pass
NPASS = PF // F                   # 2
NCLS = 10
MM_N = 512                        # one PSUM bank of fp32 per matmul

COS_M = math.cos(M)
TAN_M = math.tan(M)
LN_TAN_M = math.log(TAN_M)

f32 = mybir.dt.float32
f32r = mybir.dt.float32r
bf16 = mybir.dt.bfloat16
Alu = mybir.AluOpType
Act = mybir.ActivationFunctionType

# ln(1 + excl*e^-num) can reach e^58, past ScalarE Ln's 2^64 domain: shift
# everything by e^-LSH (folded into e_nn's bias and the Ln's bias; host
# adds LSH back per row).
LSH = 30.0
# activation-bias constants, shipped as a tiny DMA'd input (col i of "kb")
# so no gpsimd memset + all-engine barrier is needed at startup
_CONST_BIASES = (1e-30, 1e-12, LN_TAN_M, math.exp(-LSH), -LSH, 0.0)


def _patch_act_tables():
    """Force Exp and Ln onto the one table set containing both
    (natural_log_exp_and_others) so no ~2.7us table reloads occur at
    ln<->exp boundaries."""
    import concourse.hw_specs as hw_specs
    import concourse.bacc as bacc_mod

    orig = hw_specs.get_activation_tables
    if getattr(bacc_mod.get_activation_tables, "_k_patched", False):
        return
    ours = {Act.Exp, Act.Ln, Act.Copy, Act.Identity}

    def patched(module_arch):
        tables = orig(module_arch)
        target = "natural_log_exp_and_others"
        assert target in tables and ours <= tables[target], (
            target, tables.get(target))
        for name in tables:
            if name != target:
                tables[name] = tables[name] - ours
        return tables

    patched._k_patched = True
    bacc_mod.get_activation_tables = patched


def _build_graph(sym: bool):
    _patch_act_tables()
    nc = bacc.Bacc(
        "TRN2", target_bir_lowering=False, debug=False, enable_asserts=False
    )
    npairs = 5 if sym else NCLS
    x0_d = nc.dram_tensor("x0", [P, PF], bf16, kind="ExternalInput").ap()
    x1_d = nc.dram_tensor("x1", [P, PF], bf16, kind="ExternalInput").ap()
    wl0_d = nc.dram_tensor("wl0", [P, PF], bf16, kind="ExternalInput").ap()
    wl1_d = nc.dram_tensor("wl1", [P, PF], bf16, kind="ExternalInput").ap()
    # consts merged into two tensors (fewer DMA descriptors at startup):
    # wqkb = [wq | bias consts], idpm = [I | -I]
    wqkb_d = nc.dram_tensor(
        "wqkb", [P, 2 * npairs + 8], f32, kind="ExternalInput"
    ).ap()
    idpm_d = nc.dram_tensor("idpm", [P, 2 * P], f32, kind="ExternalInput").ap()
    out_d = nc.dram_tensor("out", [P, 7], f32, kind="ExternalOutput").ap()

    with tile.TileContext(nc) as tc, ExitStack() as ctx:
        _emit(ctx, tc, nc, sym, x0_d, x1_d, wl0_d, wl1_d, wqkb_d, idpm_d, out_d)
    nc.compile()
    return nc


def _emit(ctx, tc, nc, sym, x0_d, x1_d, wl0_d, wl1_d, wqkb_d, idpm_d, out_d):
    npairs = 5 if sym else NCLS
    # bufs=2 pools hold tiles whose next-pass write is emitted before this
    # pass's last read (pipelined emission would WAR-deadlock at bufs=1)
    const = ctx.enter_context(tc.tile_pool(name="const", bufs=1))
    # bufs=1 on inputs gates pass-1 loads behind pass-0's last reads, so the
    # DMA engines aren't splitting bandwidth 8 ways while pass 0 waits for
    # its first bytes
    dma_p = ctx.enter_context(tc.tile_pool(name="dma", bufs=1))
    h1 = ctx.enter_context(tc.tile_pool(name="h1", bufs=1))
    h2 = ctx.enter_context(tc.tile_pool(name="h2", bufs=2))
    h2b = ctx.enter_context(tc.tile_pool(name="h2b", bufs=1))
    zab = ctx.enter_context(tc.tile_pool(name="zab", bufs=1))
    zcp = ctx.enter_context(tc.tile_pool(name="zcp", bufs=2))
    ep = ctx.enter_context(tc.tile_pool(name="ep", bufs=3))
    etp = ctx.enter_context(tc.tile_pool(name="etp", bufs=2))
    tlA = ctx.enter_context(tc.tile_pool(name="tlA", bufs=2))
    tl = ctx.enter_context(tc.tile_pool(name="tl", bufs=1))
    psum = ctx.enter_context(tc.tile_pool(name="psum", bufs=2, space="PSUM"))

    # one-time const tiles; their DMAs are issued later (inside h12(0))
    # so their descriptors don't clog the DMA engines ahead of the first
    # x chunks. Bias constants are columns of wqkb, registered in
    # const_aps as tile-slice APs so activation reads are dep-tracked on
    # the DMA (no barrier needed).
    wqkb = const.tile([P, 2 * npairs + 8], f32, tag="wqkb")
    for i, v in enumerate(_CONST_BIASES):
        nc.const_aps.aps[(f32, v)] = wqkb[:, 2 * npairs + i : 2 * npairs + i + 1]
    idpm = const.tile([P, 2 * P], f32r, tag="idpm")
    sacc = const.tile([P, 7], f32, tag="sacc")

    def const_loads():
        nc.sync.dma_start(wqkb[:], wqkb_d[:])
        nc.sync.dma_start(idpm[:], idpm_d[:].bitcast(f32r))

    state = {}

    def h12(t, nchunk=1):
        """DMA + r^2 + 1/r, optionally sub-chunked to shorten the serial
        V->S->V warm-up chain of the very first pass."""
        x0t = dma_p.tile([P, F], bf16, tag="x0t")
        x1t = dma_p.tile([P, F], bf16, tag="x1t")
        wl0t = dma_p.tile([P, F], bf16, tag="wl0t")
        wl1t = dma_p.tile([P, F], bf16, tag="wl1t")
        fd = F // nchunk
        for k in range(nchunk):
            ck = bass.ts(t * nchunk + k, fd)
            cl = bass.ts(k, fd)
            nc.sync.dma_start(x0t[:, cl], x0_d[:, ck])
            nc.sync.dma_start(x1t[:, cl], x1_d[:, ck])
        sl = bass.ts(t, F)
        if t == 0:
            # const + wl descriptors go to the engines after the x chunks,
            # so the first compute's data is served first
            const_loads()
        nc.sync.dma_start(wl0t[:], wl0_d[:, sl])
        nc.sync.dma_start(wl1t[:], wl1_d[:, sl])

        sq0 = h1.tile([P, F], bf16, tag="sq0")
        sq1 = h1.tile([P, F], bf16, tag="sq1")
        rsq = h2.tile([P, F], bf16, tag="rsq")
        lr = h1.tile([P, F], f32, tag="lr")
        invr = h2.tile([P, F], bf16, tag="invr")
        fc = F // nchunk
        for k in range(nchunk):
            ck = bass.ts(k, fc)
            nc.vector.tensor_mul(sq0[:, ck], x0t[:, ck], x0t[:, ck])
            nc.vector.tensor_mul(sq1[:, ck], x1t[:, ck], x1t[:, ck])
            nc.vector.tensor_add(rsq[:, ck], sq0[:, ck], sq1[:, ck])
            nc.scalar.activation(lr[:, ck], rsq[:, ck], Act.Ln, bias=1e-30)
            nc.scalar.activation(invr[:, ck], lr[:, ck], Act.Exp, scale=-0.5)
        state[t] = dict(x0t=x0t, x1t=x1t, wl0t=wl0t, wl1t=wl1t, rsq=rsq,
                        invr=invr)

    def h3(t):
        """Unit vector, target dot, exp(S*t) + start of the PSUM chain,
        then r/soft accums (kept after tt so ScalarE unblocks early)."""
        st = state[t]
        x0t, x1t, invr, rsq = st["x0t"], st["x1t"], st["invr"], st["rsq"]
        y0 = h2b.tile([P, F], bf16, tag="y0")
        nc.vector.tensor_mul(y0[:], x0t[:], invr[:])
        y1 = h2b.tile([P, F], bf16, tag="y1")
        nc.vector.tensor_mul(y1[:], x1t[:], invr[:])
        t1 = h1.tile([P, F], bf16, tag="t1")
        nc.vector.tensor_mul(t1[:], y0[:], st["wl0t"][:])
        t2 = h1.tile([P, F], bf16, tag="t2")
        nc.vector.tensor_mul(t2[:], y1[:], st["wl1t"][:])
        tt = h2b.tile([P, F], bf16, tag="tt")
        nc.vector.tensor_add(tt[:], t1[:], t2[:])

        e_t = etp.tile([P, F], f32r, tag="et")
        nc.scalar.activation(e_t[:], tt[:], Act.Exp, scale=S)
        es_ps = psum.tile([P, F], f32, tag="es")
        for k in range(F // MM_N):
            ck = bass.ts(k, MM_N)
            nc.tensor.matmul(
                es_ps[:, ck], idpm[:, P:], e_t[:, ck], start=True, stop=False
            )

        # r on the otherwise-idle GpSimd engine (feeds only the soft sums)
        rs = h1.tile([P, F], bf16, tag="rs")
        nc.gpsimd.tensor_mul(rs[:], rsq[:], invr[:])
        state[t].update(y0=y0, y1=y1, tt=tt, rs=rs, es=es_ps)

    def soft_accums(t):
        """Soft-loss partial sums; spliced late in the class loop so they
        don't block the dot/exp critical path. accum_out =
        reduce<op1=add>(op0 result): Sa = sum(min(r,1.5)),
        Sb = sum(max(r,2)); host: Sb - Sa - 0.5*N."""
        rs = state[t]["rs"]
        trash_a = h1.tile([P, F], bf16, tag="trash_a")
        nc.vector.tensor_scalar(
            trash_a[:], rs[:], 1.5, None, Alu.min, Alu.add,
            accum_out=sacc[:, 3 * t + 1 : 3 * t + 2],
        )
        trash_b = h1.tile([P, F], bf16, tag="trash_b")
        nc.vector.tensor_scalar(
            trash_b[:], rs[:], 2.0, None, Alu.max, Alu.add,
            accum_out=sacc[:, 3 * t + 2 : 3 * t + 3],
        )

    def tail_pre(t):
        """clip(t), 1-t^2 -- only needs tt; spliced into the class loop."""
        st = state[t]
        tcl = tlA.tile([P, F], bf16, tag="tcl")
        nc.vector.tensor_scalar(tcl[:], st["tt"][:], 1.0, -1.0, Alu.min, Alu.max)
        tsq = tl.tile([P, F], bf16, tag="tsq")
        nc.vector.tensor_mul(tsq[:], tcl[:], tcl[:])
        u = tlA.tile([P, F], bf16, tag="u")
        nc.vector.tensor_scalar(u[:], tsq[:], -1.0, 1.0, Alu.mult, Alu.add)
        state[t].update(tcl=tcl, u=u)

    def classes(t, spliced):
        """Class dot/exp/matmul loop for pass t; `spliced` maps a class
        index to a list of closures emitted after that class (the previous
        pass's tail, interleaved to hide V<->S ping-pong)."""
        st = state[t]
        y0, y1, es_ps = st["y0"], st["y1"], st["es"]
        n_exp = 2 if sym else 1
        for c in range(npairs):
            za = zab.tile([P, F], bf16, tag="za")
            nc.vector.tensor_scalar(
                za[:], y0[:], wqkb[:, 2 * c : 2 * c + 1], None, Alu.mult
            )
            zb = zab.tile([P, F], bf16, tag="zb")
            nc.vector.tensor_scalar(
                zb[:], y1[:], wqkb[:, 2 * c + 1 : 2 * c + 2], None, Alu.mult
            )
            zc = zcp.tile([P, F], bf16, tag="zc")
            nc.vector.tensor_add(zc[:], za[:], zb[:])
            for j in range(n_exp):
                e = ep.tile([P, F], f32r, tag="e")
                nc.scalar.activation(e[:], zc[:], Act.Exp, scale=(S, -S)[j])
                last = (c == npairs - 1) and (j == n_exp - 1)
                for k in range(F // MM_N):
                    ck = bass.ts(k, MM_N)
                    nc.tensor.matmul(
                        es_ps[:, ck], idpm[:, :P], e[:, ck], start=False, stop=last
                    )
            for fn in spliced.get(c, ()):
                fn()

    def tail_post_ops(t, parts):
        """Stage closures for pass t's tail over free-dim sub-ranges.
        parts: list of (offset, length, lden_col). Returns per-stage lists
        of closures (each stage covers all parts, so a later part's
        ScalarE leg hides an earlier part's VectorE leg and vice versa)."""
        st = state[t]
        # one allocation per tag per pass; parts write disjoint slices
        lnu = tl.tile([P, F], bf16, tag="lnu")
        sqru = tl.tile([P, F], bf16, tag="sqru")
        nump = tl.tile([P, F], bf16, tag="nump")
        e_nn = tl.tile([P, F], f32, tag="enn")
        w = tl.tile([P, F], f32, tag="w")
        trash = tl.tile([P, F], bf16, tag="trash_l")

        def mk(stage, p):
            off, ln, col = parts[p]
            fs = slice(off, off + ln)

            def s_lnu():
                nc.scalar.activation(lnu[:, fs], st["u"][:, fs], Act.Ln, bias=1e-12)
                nc.scalar.activation(
                    sqru[:, fs], lnu[:, fs], Act.Exp, scale=0.5, bias=LN_TAN_M
                )

            def v_nump():
                nc.vector.tensor_tensor(
                    nump[:, fs], st["tcl"][:, fs], sqru[:, fs], Alu.subtract
                )

            def s_enn():
                nc.scalar.activation(
                    e_nn[:, fs], nump[:, fs], Act.Exp, scale=-S * COS_M, bias=-LSH
                )

            def v_w():
                nc.vector.tensor_mul(w[:, fs], st["es"][:, fs], e_nn[:, fs])

            def s_lnacc():
                nc.scalar.activation(
                    trash[:, fs], w[:, fs], Act.Ln, bias=math.exp(-LSH),
                    accum_out=sacc[:, col : col + 1],
                )

            return [s_lnu, v_nump, s_enn, v_w, s_lnacc][stage]

        return [[mk(stage, p) for p in range(len(parts))] for stage in range(5)]

    # ---- software-pipelined emission ----
    # h12(1) must come after h3(0): with bufs=1 input tiles, pass-1 DMAs
    # wait on pass-0's last reads (y0/t1), which must precede them in the
    # engine queues
    h12(0, nchunk=2)
    h3(0)
    h12(1)
    classes(0, {0: [lambda: tail_pre(0)],
                2: [lambda: soft_accums(0)]})
    h3(1)
    spliced = {0: [lambda: tail_pre(1)],
               3: [lambda: soft_accums(1)]}
    stages = tail_post_ops(0, [(0, F, 0)])
    for i, fns in enumerate(stages):
        spliced.setdefault(min(1 + i, npairs - 1), []).extend(fns)
    classes(1, spliced)
    # final tail: two interleaved half-chunks to overlap the V<->S chain
    fstages = tail_post_ops(1, [(0, F // 2, 3), (F // 2, F // 2, 6)])
    for fns in fstages:
        for fn in fns:
            fn()

    nc.sync.dma_start(out_d[:], sacc[:])


_NC_CACHE = {}


def _get_graph(sym: bool):
    if sym not in _NC_CACHE:
        _NC_CACHE[sym] = _build_graph(sym)
    return _NC_CACHE[sym]


def kernel(x, labels, weight):
    import ml_dtypes

    BF = ml_dtypes.bfloat16
    x = np.asarray(x, dtype=np.float32)
    labels = np.asarray(labels).astype(np.int64)
    w = np.asarray(weight, dtype=np.float32)

    wb = w.astype(BF)
    sym = w.shape[0] == NCLS and np.array_equal(
        wb[NCLS // 2 :], -wb[: NCLS // 2]
    )
    npairs = 5 if sym else NCLS
    nc = _get_graph(sym)

    wqkb_row = np.zeros(2 * npairs + 8, dtype=np.float32)
    wqkb_row[: 2 * npairs] = wb[:npairs].astype(np.float32).reshape(-1)
    wqkb_row[2 * npairs : 2 * npairs + len(_CONST_BIASES)] = _CONST_BIASES
    wqkb = np.ascontiguousarray(np.tile(wqkb_row.reshape(1, -1), (P, 1)))
    idpm = np.ascontiguousarray(
        np.concatenate([np.eye(P, dtype=np.float32), -np.eye(P, dtype=np.float32)],
                       axis=1)
    )

    wl = wb[labels]                      # [N, 2] bf16 gather on host
    x0 = x[:, 0].astype(BF)
    x1 = x[:, 1].astype(BF)

    in_maps = []
    for i in range(N_CORES):
        sl = slice(i * NC_ROWS, (i + 1) * NC_ROWS)
        in_maps.append(
            {
                "x0": np.ascontiguousarray(x0[sl]).reshape(P, PF),
                "x1": np.ascontiguousarray(x1[sl]).reshape(P, PF),
                "wl0": np.ascontiguousarray(wl[sl, 0]).reshape(P, PF),
                "wl1": np.ascontiguousarray(wl[sl, 1]).reshape(P, PF),
                "wqkb": wqkb,
                "idpm": idpm,
            }
        )

    trace = os.environ.get("KTRACE", "0") == "1"
    res = run_bass_kernel_spmd(nc, in_maps, core_ids=list(range(N_CORES)), trace=trace)
    if getattr(res, "exec_time_ns", None):
        print(f"HW exec time: {res.exec_time_ns} ns")

    lden_sum = 0.0
    soft_sum = -0.5 * N
    for i in range(N_CORES):
        o = np.asarray(res.results[i]["out"], dtype=np.float64)
        lden_sum += o[:, 0].sum() + o[:, 3].sum() + o[:, 6].sum()
        soft_sum += (o[:, 2].sum() - o[:, 1].sum()) + (o[:, 5].sum() - o[:, 4].sum())

    loss = lden_sum / N + LSH + (LBDA / 2.0) * (soft_sum / N)
    return np.float32(loss)


if __name__ == "__main__":
    rng = np.random.default_rng(0)
    x = rng.standard_normal((N, 2), dtype=np.float32)
    labels = rng.integers(0, 10, size=(N,)).astype(np.int64)
    w = np.array(
        [[1, 0], [0.809, 0.588], [0.309, 0.951], [-0.309, 0.951], [-0.809, 0.588],
         [-1, 0], [-0.809, -0.588], [-0.309, -0.951], [0.309, -0.951], [0.809, -0.588]],
        dtype=np.float32,
    )
    print(kernel(x, labels, w))


# revision 47
# speedup vs baseline: 1.0810x; 1.0485x over previous
"""
AngularPenaltySMLoss on 8 Trainium2 NeuronCores, pure data parallel.

Math (reference):
    r = ||x_i||;  soft = relu(1.5 - r) + relu(r - 2)
    xn = x / max(r, eps);  wf = xn @ W.T   (W is [10, 2])
    t = wf[i, label_i];  num = S*cos(arccos(clip(t)) + M)
    den = exp(num) + sum_c exp(S*wf_c) - exp(S*t)
    loss = -mean(num - log(den)) + LBDA*mean(soft)/2

Kernel strategy (per core, 524288 rows as [128 x 4096], two passes of
F=2048, all VectorE elementwise in bf16):
  - Host pre-gathers the target-class weights wl = W_bf16[label] and ships
    them as two bf16 planes: the target dot tt = y0*wl0 + y1*wl1 is then
    BIT-IDENTICAL to the class dot z_label (same inputs, same ops), so
    excl_sum = sum_c exp(S z_c) - exp(S tt) cancels exactly -- no
    per-class label masks needed at all.
  - W has w[c+5] == -w[c] exactly (checked on host): only 5 class dots are
    computed; exp(+S z) and exp(-S z) come from two ScalarE activations on
    the same tile (scale=+-S). Falls back to a 10-dot graph otherwise.
  - sum_c exp(.) - exp(S tt) accumulates on TensorE via identity-matmul
    chains into PSUM (idm = -I for the target term).
  - Whole tail folds into one accumulated Ln:
      -L = ln(1 + excl * e^{-num}) summed by the activation's accum_out.
    num is never materialized or summed; exp(-num) comes from one Exp.
  - softloss partial sums ride free on tensor_scalar accum_out:
      a = min(r,1.5)-1.5 (= -relu(1.5-r)),  b = max(r,2)-2 (= relu(r-2)).
  - Host sums the 8 cores' [128, 6] partial tiles.
"""

import math
import os
import sys

import numpy as np

for _p in ("/opt/trn_rl_repo", "/root/.axon_site/_ro/trn_rl_repo"):
    if os.path.isdir(_p) and _p not in sys.path:
        sys.path.insert(0, _p)

from contextlib import ExitStack

from concourse import bacc, bass, tile
from concourse import mybir
from concourse.bass_utils import run_bass_kernel_spmd

# ---- problem constants (hardcoded; kernel.py must be self-contained) ----
S = 30.0
M = 0.5
LBDA = 1.0
N = 4_194_304
N_CORES = 8
P = 128
NC_ROWS = N // N_CORES            # 524288 rows per core
PF = NC_ROWS // P                 # 4096 per partition
F = 2048                          # free-dim per pass
NPASS = PF // F                   # 2
NCLS = 10
MM_N = 512                        # one PSUM bank of fp32 per matmul

COS_M = math.cos(M)
TAN_M = math.tan(M)
LN_TAN_M = math.log(TAN_M)

f32 = mybir.dt.float32
f32r = mybir.dt.float32r
bf16 = mybir.dt.bfloat16
Alu = mybir.AluOpType
Act = mybir.ActivationFunctionType

# ln(1 + excl*e^-num) can reach e^58, past ScalarE Ln's 2^64 domain: shift
# everything by e^-LSH (folded into e_nn's bias and the Ln's bias; host
# adds LSH back per row).
LSH = 30.0
# activation-bias constants, shipped as a tiny DMA'd input (col i of "kb")
# so no gpsimd memset + all-engine barrier is needed at startup
_CONST_BIASES = (1e-30, 1e-12, LN_TAN_M, math.exp(-LSH), -LSH, 0.0)


def _patch_act_tables():
    """Force Exp and Ln onto the one table set containing both
    (natural_log_exp_and_others) so no ~2.7us table reloads occur at
    ln<->exp boundaries."""
    import concourse.hw_specs as hw_specs
    import concourse.bacc as bacc_mod

    orig = hw_specs.get_activation_tables
    if getattr(bacc_mod.get_activation_tables, "_k_patched", False):
        return
    ours = {Act.Exp, Act.Ln, Act.Copy, Act.Identity}

    def patched(module_arch):
        tables = orig(module_arch)
        target = "natural_log_exp_and_others"
        assert target in tables and ours <= tables[target], (
            target, tables.get(target))
        for name in tables:
            if name != target:
                tables[name] = tables[name] - ours
        return tables

    patched._k_patched = True
    bacc_mod.get_activation_tables = patched


def _build_graph(sym: bool):
    _patch_act_tables()
    nc = bacc.Bacc(
        "TRN2", target_bir_lowering=False, debug=False, enable_asserts=False
    )
    npairs = 5 if sym else NCLS
    x0_d = nc.dram_tensor("x0", [P, PF], bf16, kind="ExternalInput").ap()
    x1_d = nc.dram_tensor("x1", [P, PF], bf16, kind="ExternalInput").ap()
    wl0_d = nc.dram_tensor("wl0", [P, PF], bf16, kind="ExternalInput").ap()
    wl1_d = nc.dram_tensor("wl1", [P, PF], bf16, kind="ExternalInput").ap()
    # consts merged into two tensors (fewer DMA descriptors at startup):
    # wqkb = [wq | bias consts], idpm = [I | -I]
    wqkb_d = nc.dram_tensor(
        "wqkb", [P, 2 * npairs + 8], f32, kind="ExternalInput"
    ).ap()
    idpm_d = nc.dram_tensor("idpm", [P, 2 * P], f32, kind="ExternalInput").ap()
    out_d = nc.dram_tensor("out", [P, 7], f32, kind="ExternalOutput").ap()

    with tile.TileContext(nc) as tc, ExitStack() as ctx:
        _emit(ctx, tc, nc, sym, x0_d, x1_d, wl0_d, wl1_d, wqkb_d, idpm_d, out_d)
    nc.compile()
    return nc


def _emit(ctx, tc, nc, sym, x0_d, x1_d, wl0_d, wl1_d, wqkb_d, idpm_d, out_d):
    npairs = 5 if sym else NCLS
    # bufs=2 pools hold tiles whose next-pass write is emitted before this
    # pass's last read (pipelined emission would WAR-deadlock at bufs=1)
    const = ctx.enter_context(tc.tile_pool(name="const", bufs=1))
    # bufs=1 on inputs gates pass-1 loads behind pass-0's last reads, so the
    # DMA engines aren't splitting bandwidth 8 ways while pass 0 waits for
    # its first bytes
    dma_p = ctx.enter_context(tc.tile_pool(name="dma", bufs=1))
    h1 = ctx.enter_context(tc.tile_pool(name="h1", bufs=1))
    h2 = ctx.enter_context(tc.tile_pool(name="h2", bufs=2))
    h2b = ctx.enter_context(tc.tile_pool(name="h2b", bufs=1))
    zab = ctx.enter_context(tc.tile_pool(name="zab", bufs=1))
    zcp = ctx.enter_context(tc.tile_pool(name="zcp", bufs=2))
    ep = ctx.enter_context(tc.tile_pool(name="ep", bufs=3))
    etp = ctx.enter_context(tc.tile_pool(name="etp", bufs=2))
    tlA = ctx.enter_context(tc.tile_pool(name="tlA", bufs=2))
    tl = ctx.enter_context(tc.tile_pool(name="tl", bufs=1))
    psum = ctx.enter_context(tc.tile_pool(name="psum", bufs=2, space="PSUM"))

    # one-time const tiles; their DMAs are issued later (inside h12(0))
    # so their descriptors don't clog the DMA engines ahead of the first
    # x chunks. Bias constants are columns of wqkb, registered in
    # const_aps as tile-slice APs so activation reads are dep-tracked on
    # the DMA (no barrier needed).
    wqkb = const.tile([P, 2 * npairs + 8], f32, tag="wqkb")
    for i, v in enumerate(_CONST_BIASES):
        nc.const_aps.aps[(f32, v)] = wqkb[:, 2 * npairs + i : 2 * npairs + i + 1]
    idpm = const.tile([P, 2 * P], f32r, tag="idpm")
    sacc = const.tile([P, 7], f32, tag="sacc")

    def const_loads():
        nc.sync.dma_start(wqkb[:], wqkb_d[:])
        nc.sync.dma_start(idpm[:], idpm_d[:].bitcast(f32r))

    state = {}

    def h12(t, nchunk=1):
        """DMA + r^2 + 1/r, optionally sub-chunked to shorten the serial
        V->S->V warm-up chain of the very first pass."""
        x0t = dma_p.tile([P, F], bf16, tag="x0t")
        x1t = dma_p.tile([P, F], bf16, tag="x1t")
        wl0t = dma_p.tile([P, F], bf16, tag="wl0t")
        wl1t = dma_p.tile([P, F], bf16, tag="wl1t")
        fd = F // nchunk
        for k in range(nchunk):
            ck = bass.ts(t * nchunk + k, fd)
            cl = bass.ts(k, fd)
            nc.sync.dma_start(x0t[:, cl], x0_d[:, ck])
            nc.sync.dma_start(x1t[:, cl], x1_d[:, ck])
        sl = bass.ts(t, F)
        if t == 0:
            # const + wl descriptors go to the engines after the x chunks,
            # so the first compute's data is served first
            const_loads()
        nc.sync.dma_start(wl0t[:], wl0_d[:, sl])
        nc.sync.dma_start(wl1t[:], wl1_d[:, sl])

        sq0 = h1.tile([P, F], bf16, tag="sq0")
        sq1 = h1.tile([P, F], bf16, tag="sq1")
        rsq = h2.tile([P, F], bf16, tag="rsq")
        lr = h1.tile([P, F], f32, tag="lr")
        invr = h2.tile([P, F], bf16, tag="invr")
        fc = F // nchunk
        for k in range(nchunk):
            ck = bass.ts(k, fc)
            nc.vector.tensor_mul(sq0[:, ck], x0t[:, ck], x0t[:, ck])
            nc.vector.tensor_mul(sq1[:, ck], x1t[:, ck], x1t[:, ck])
            nc.vector.tensor_add(rsq[:, ck], sq0[:, ck], sq1[:, ck])
            nc.scalar.activation(lr[:, ck], rsq[:, ck], Act.Ln, bias=1e-30)
            nc.scalar.activation(invr[:, ck], lr[:, ck], Act.Exp, scale=-0.5)
        state[t] = dict(x0t=x0t, x1t=x1t, wl0t=wl0t, wl1t=wl1t, rsq=rsq,
                        invr=invr)

    def h3a(t):
        """Unit vector only -- the minimum V work before class dots."""
        st = state[t]
        y0 = h2b.tile([P, F], bf16, tag="y0")
        nc.vector.tensor_mul(y0[:], st["x0t"][:], st["invr"][:])
        y1 = h2b.tile([P, F], bf16, tag="y1")
        nc.vector.tensor_mul(y1[:], st["x1t"][:], st["invr"][:])
        es_ps = psum.tile([P, F], f32, tag="es")
        state[t].update(y0=y0, y1=y1, es=es_ps)

    def h3b(t):
        """Target dot + e_t (closes the PSUM chain with stop=True) + r.
        Emitted after the class dots so VectorE doesn't detour while
        ScalarE is consuming the dot stream."""
        st = state[t]
        y0, y1, es_ps = st["y0"], st["y1"], st["es"]
        t1 = h1.tile([P, F], bf16, tag="t1")
        nc.vector.tensor_mul(t1[:], y0[:], st["wl0t"][:])
        t2 = h1.tile([P, F], bf16, tag="t2")
        nc.vector.tensor_mul(t2[:], y1[:], st["wl1t"][:])
        tt = h2b.tile([P, F], bf16, tag="tt")
        nc.vector.tensor_add(tt[:], t1[:], t2[:])

        e_t = etp.tile([P, F], f32r, tag="et")
        nc.scalar.activation(e_t[:], tt[:], Act.Exp, scale=S)
        for k in range(F // MM_N):
            ck = bass.ts(k, MM_N)
            nc.tensor.matmul(
                es_ps[:, ck], idpm[:, P:], e_t[:, ck], start=False, stop=True
            )

        # r on the otherwise-idle GpSimd engine (feeds only the soft sums)
        rs = h1.tile([P, F], bf16, tag="rs")
        nc.gpsimd.tensor_mul(rs[:], st["rsq"][:], st["invr"][:])
        state[t].update(tt=tt, rs=rs)

    def soft_accums(t):
        """Soft-loss partial sums; spliced late in the class loop so they
        don't block the dot/exp critical path. accum_out =
        reduce<op1=add>(op0 result): Sa = sum(min(r,1.5)),
        Sb = sum(max(r,2)); host: Sb - Sa - 0.5*N."""
        rs = state[t]["rs"]
        trash_a = h1.tile([P, F], bf16, tag="trash_a")
        nc.vector.tensor_scalar(
            trash_a[:], rs[:], 1.5, None, Alu.min, Alu.add,
            accum_out=sacc[:, 3 * t + 1 : 3 * t + 2],
        )
        trash_b = h1.tile([P, F], bf16, tag="trash_b")
        nc.vector.tensor_scalar(
            trash_b[:], rs[:], 2.0, None, Alu.max, Alu.add,
            accum_out=sacc[:, 3 * t + 2 : 3 * t + 3],
        )

    def tail_pre(t):
        """clip(t), 1-t^2 -- only needs tt; spliced into the class loop."""
        st = state[t]
        tcl = tlA.tile([P, F], bf16, tag="tcl")
        nc.vector.tensor_scalar(tcl[:], st["tt"][:], 1.0, -1.0, Alu.min, Alu.max)
        tsq = tl.tile([P, F], bf16, tag="tsq")
        nc.vector.tensor_mul(tsq[:], tcl[:], tcl[:])
        u = tlA.tile([P, F], bf16, tag="u")
        nc.vector.tensor_scalar(u[:], tsq[:], -1.0, 1.0, Alu.mult, Alu.add)
        state[t].update(tcl=tcl, u=u)

    def classes(t, spliced):
        """Class dot/exp/matmul loop for pass t; `spliced` maps a class
        index to closures emitted after that class (the previous pass's
        tail stages -- small V legs + S legs that fill ScalarE's queue)."""
        st = state[t]
        y0, y1, es_ps = st["y0"], st["y1"], st["es"]
        n_exp = 2 if sym else 1
        for c in range(npairs):
            za = zab.tile([P, F], bf16, tag="za")
            nc.vector.tensor_scalar(
                za[:], y0[:], wqkb[:, 2 * c : 2 * c + 1], None, Alu.mult
            )
            zb = zab.tile([P, F], bf16, tag="zb")
            nc.vector.tensor_scalar(
                zb[:], y1[:], wqkb[:, 2 * c + 1 : 2 * c + 2], None, Alu.mult
            )
            zc = zcp.tile([P, F], bf16, tag="zc")
            nc.vector.tensor_add(zc[:], za[:], zb[:])
            for j in range(n_exp):
                e = ep.tile([P, F], f32r, tag="e")
                nc.scalar.activation(e[:], zc[:], Act.Exp, scale=(S, -S)[j])
                start = (c == 0) and (j == 0)
                for k in range(F // MM_N):
                    ck = bass.ts(k, MM_N)
                    nc.tensor.matmul(
                        es_ps[:, ck], idpm[:, :P], e[:, ck], start=start, stop=False
                    )
            for fn in spliced.get(c, ()):
                fn()

    def tail_post_ops(t, parts):
        """Stage closures for pass t's tail over free-dim sub-ranges.
        parts: list of (offset, length, lden_col). Returns per-stage lists
        of closures (each stage covers all parts, so a later part's
        ScalarE leg hides an earlier part's VectorE leg and vice versa)."""
        st = state[t]
        # one allocation per tag per pass; parts write disjoint slices
        lnu = tl.tile([P, F], bf16, tag="lnu")
        sqru = tl.tile([P, F], bf16, tag="sqru")
        nump = tl.tile([P, F], bf16, tag="nump")
        e_nn = tl.tile([P, F], f32, tag="enn")
        w = tl.tile([P, F], f32, tag="w")
        trash = tl.tile([P, F], bf16, tag="trash_l")

        def mk(stage, p):
            off, ln, col = parts[p]
            fs = slice(off, off + ln)

            def s_lnu():
                nc.scalar.activation(lnu[:, fs], st["u"][:, fs], Act.Ln, bias=1e-12)
                nc.scalar.activation(
                    sqru[:, fs], lnu[:, fs], Act.Exp, scale=0.5, bias=LN_TAN_M
                )

            def v_nump():
                nc.vector.tensor_tensor(
                    nump[:, fs], st["tcl"][:, fs], sqru[:, fs], Alu.subtract
                )

            def s_enn():
                nc.scalar.activation(
                    e_nn[:, fs], nump[:, fs], Act.Exp, scale=-S * COS_M, bias=-LSH
                )

            def v_w():
                nc.vector.tensor_mul(w[:, fs], st["es"][:, fs], e_nn[:, fs])

            def s_lnacc():
                nc.scalar.activation(
                    trash[:, fs], w[:, fs], Act.Ln, bias=math.exp(-LSH),
                    accum_out=sacc[:, col : col + 1],
                )

            return [s_lnu, v_nump, s_enn, v_w, s_lnacc][stage]

        return [[mk(stage, p) for p in range(len(parts))] for stage in range(5)]

    # ---- software-pipelined emission ----
    # Rule: while ScalarE is consuming a pass's dot stream (2 exps per
    # dot), VectorE must not detour -- all non-dot V work is emitted
    # after that pass's dots.
    h12(0, nchunk=2)
    h3a(0)
    classes(0, {})
    h3b(0)
    tail_pre(0)
    h12(1)
    h3a(1)
    spliced = {}
    stages = tail_post_ops(0, [(0, F, 0)])
    for i, fns in enumerate(stages):
        spliced.setdefault(min(1 + i, npairs - 1), []).extend(fns)
    classes(1, spliced)
    h3b(1)
    tail_pre(1)
    soft_accums(0)
    soft_accums(1)
    # final tail: two interleaved half-chunks to overlap the V<->S chain
    fstages = tail_post_ops(1, [(0, F // 2, 3), (F // 2, F // 2, 6)])
    for fns in fstages:
        for fn in fns:
            fn()

    nc.sync.dma_start(out_d[:], sacc[:])


_NC_CACHE = {}


def _get_graph(sym: bool):
    if sym not in _NC_CACHE:
        _NC_CACHE[sym] = _build_graph(sym)
    return _NC_CACHE[sym]


def kernel(x, labels, weight):
    import ml_dtypes

    BF = ml_dtypes.bfloat16
    x = np.asarray(x, dtype=np.float32)
    labels = np.asarray(labels).astype(np.int64)
    w = np.asarray(weight, dtype=np.float32)

    wb = w.astype(BF)
    sym = w.shape[0] == NCLS and np.array_equal(
        wb[NCLS // 2 :], -wb[: NCLS // 2]
    )
    npairs = 5 if sym else NCLS
    nc = _get_graph(sym)

    wqkb_row = np.zeros(2 * npairs + 8, dtype=np.float32)
    wqkb_row[: 2 * npairs] = wb[:npairs].astype(np.float32).reshape(-1)
    wqkb_row[2 * npairs : 2 * npairs + len(_CONST_BIASES)] = _CONST_BIASES
    wqkb = np.ascontiguousarray(np.tile(wqkb_row.reshape(1, -1), (P, 1)))
    idpm = np.ascontiguousarray(
        np.concatenate([np.eye(P, dtype=np.float32), -np.eye(P, dtype=np.float32)],
                       axis=1)
    )

    wl = wb[labels]                      # [N, 2] bf16 gather on host
    x0 = x[:, 0].astype(BF)
    x1 = x[:, 1].astype(BF)

    in_maps = []
    for i in range(N_CORES):
        sl = slice(i * NC_ROWS, (i + 1) * NC_ROWS)
        in_maps.append(
            {
                "x0": np.ascontiguousarray(x0[sl]).reshape(P, PF),
                "x1": np.ascontiguousarray(x1[sl]).reshape(P, PF),
                "wl0": np.ascontiguousarray(wl[sl, 0]).reshape(P, PF),
                "wl1": np.ascontiguousarray(wl[sl, 1]).reshape(P, PF),
                "wqkb": wqkb,
                "idpm": idpm,
            }
        )

    trace = os.environ.get("KTRACE", "0") == "1"
    res = run_bass_kernel_spmd(nc, in_maps, core_ids=list(range(N_CORES)), trace=trace)
    if getattr(res, "exec_time_ns", None):
        print(f"HW exec time: {res.exec_time_ns} ns")

    lden_sum = 0.0
    soft_sum = -0.5 * N
    for i in range(N_CORES):
        o = np.asarray(res.results[i]["out"], dtype=np.float64)
        lden_sum += o[:, 0].sum() + o[:, 3].sum() + o[:, 6].sum()
        soft_sum += (o[:, 2].sum() - o[:, 1].sum()) + (o[:, 5].sum() - o[:, 4].sum())

    loss = lden_sum / N + LSH + (LBDA / 2.0) * (soft_sum / N)
    return np.float32(loss)


if __name__ == "__main__":
    rng = np.random.default_rng(0)
    x = rng.standard_normal((N, 2), dtype=np.float32)
    labels = rng.integers(0, 10, size=(N,)).astype(np.int64)
    w = np.array(
        [[1, 0], [0.809, 0.588], [0.309, 0.951], [-0.309, 0.951], [-0.809, 0.588],
         [-1, 0], [-0.809, -0.588], [-0.309, -0.951], [0.309, -0.951], [0.809, -0.588]],
        dtype=np.float32,
    )
    print(kernel(x, labels, w))


# revision 51
# speedup vs baseline: 1.0895x; 1.0078x over previous
"""
AngularPenaltySMLoss on 8 Trainium2 NeuronCores, pure data parallel.

Math (reference):
    r = ||x_i||;  soft = relu(1.5 - r) + relu(r - 2)
    xn = x / max(r, eps);  wf = xn @ W.T   (W is [10, 2])
    t = wf[i, label_i];  num = S*cos(arccos(clip(t)) + M)
    den = exp(num) + sum_c exp(S*wf_c) - exp(S*t)
    loss = -mean(num - log(den)) + LBDA*mean(soft)/2

Kernel strategy (per core, 524288 rows as [128 x 4096], two passes of
F=2048, all VectorE elementwise in bf16):
  - Host pre-gathers the target-class weights wl = W_bf16[label] and ships
    them as two bf16 planes: the target dot tt = y0*wl0 + y1*wl1 is then
    BIT-IDENTICAL to the class dot z_label (same inputs, same ops), so
    excl_sum = sum_c exp(S z_c) - exp(S tt) cancels exactly -- no
    per-class label masks needed at all.
  - W has w[c+5] == -w[c] exactly (checked on host): only 5 class dots are
    computed; exp(+S z) and exp(-S z) come from two ScalarE activations on
    the same tile (scale=+-S). Falls back to a 10-dot graph otherwise.
  - sum_c exp(.) - exp(S tt) accumulates on TensorE via identity-matmul
    chains into PSUM (idm = -I for the target term).
  - Whole tail folds into one accumulated Ln:
      -L = ln(1 + excl * e^{-num}) summed by the activation's accum_out.
    num is never materialized or summed; exp(-num) comes from one Exp.
  - softloss partial sums ride free on tensor_scalar accum_out:
      a = min(r,1.5)-1.5 (= -relu(1.5-r)),  b = max(r,2)-2 (= relu(r-2)).
  - Host sums the 8 cores' [128, 6] partial tiles.
"""

import math
import os
import sys

import numpy as np

for _p in ("/opt/trn_rl_repo", "/root/.axon_site/_ro/trn_rl_repo"):
    if os.path.isdir(_p) and _p not in sys.path:
        sys.path.insert(0, _p)

from contextlib import ExitStack

from concourse import bacc, bass, tile
from concourse import mybir
from concourse.bass_utils import run_bass_kernel_spmd

# ---- problem constants (hardcoded; kernel.py must be self-contained) ----
S = 30.0
M = 0.5
LBDA = 1.0
N = 4_194_304
N_CORES = 8
P = 128
NC_ROWS = N // N_CORES            # 524288 rows per core
PF = NC_ROWS // P                 # 4096 per partition
F = 2048                          # free-dim per pass
NPASS = PF // F                   # 2
NCLS = 10
MM_N = 512                        # one PSUM bank of fp32 per matmul

COS_M = math.cos(M)
TAN_M = math.tan(M)
LN_TAN_M = math.log(TAN_M)

f32 = mybir.dt.float32
f32r = mybir.dt.float32r
bf16 = mybir.dt.bfloat16
Alu = mybir.AluOpType
Act = mybir.ActivationFunctionType

# ln(1 + excl*e^-num) can reach e^58, past ScalarE Ln's 2^64 domain: shift
# everything by e^-LSH (folded into e_nn's bias and the Ln's bias; host
# adds LSH back per row).
LSH = 30.0
# activation-bias constants, shipped as a tiny DMA'd input (col i of "kb")
# so no gpsimd memset + all-engine barrier is needed at startup
_CONST_BIASES = (1e-30, 1e-12, LN_TAN_M, math.exp(-LSH), -LSH, 0.0)


def _patch_act_tables():
    """Force Exp and Ln onto the one table set containing both
    (natural_log_exp_and_others) so no ~2.7us table reloads occur at
    ln<->exp boundaries."""
    import concourse.hw_specs as hw_specs
    import concourse.bacc as bacc_mod

    orig = hw_specs.get_activation_tables
    if getattr(bacc_mod.get_activation_tables, "_k_patched", False):
        return
    ours = {Act.Exp, Act.Ln, Act.Copy, Act.Identity}

    def patched(module_arch):
        tables = orig(module_arch)
        target = "natural_log_exp_and_others"
        assert target in tables and ours <= tables[target], (
            target, tables.get(target))
        for name in tables:
            if name != target:
                tables[name] = tables[name] - ours
        return tables

    patched._k_patched = True
    bacc_mod.get_activation_tables = patched


def _build_graph(sym: bool):
    _patch_act_tables()
    nc = bacc.Bacc(
        "TRN2", target_bir_lowering=False, debug=False, enable_asserts=False
    )
    npairs = 5 if sym else NCLS
    x0_d = nc.dram_tensor("x0", [P, PF], bf16, kind="ExternalInput").ap()
    x1_d = nc.dram_tensor("x1", [P, PF], bf16, kind="ExternalInput").ap()
    wl0_d = nc.dram_tensor("wl0", [P, PF], bf16, kind="ExternalInput").ap()
    wl1_d = nc.dram_tensor("wl1", [P, PF], bf16, kind="ExternalInput").ap()
    # consts merged into two tensors (fewer DMA descriptors at startup):
    # wqkb = [wq | bias consts], idpm = [I | -I]
    wqkb_d = nc.dram_tensor(
        "wqkb", [P, 2 * npairs + 8], f32, kind="ExternalInput"
    ).ap()
    idpm_d = nc.dram_tensor("idpm", [P, 2 * P], f32, kind="ExternalInput").ap()
    out_d = nc.dram_tensor("out", [P, 7], f32, kind="ExternalOutput").ap()

    with tile.TileContext(nc) as tc, ExitStack() as ctx:
        _emit(ctx, tc, nc, sym, x0_d, x1_d, wl0_d, wl1_d, wqkb_d, idpm_d, out_d)
    nc.compile()
    return nc


def _emit(ctx, tc, nc, sym, x0_d, x1_d, wl0_d, wl1_d, wqkb_d, idpm_d, out_d):
    npairs = 5 if sym else NCLS
    # bufs=2 pools hold tiles whose next-pass write is emitted before this
    # pass's last read (pipelined emission would WAR-deadlock at bufs=1)
    const = ctx.enter_context(tc.tile_pool(name="const", bufs=1))
    # bufs=1 on inputs gates pass-1 loads behind pass-0's last reads, so the
    # DMA engines aren't splitting bandwidth 8 ways while pass 0 waits for
    # its first bytes
    dma_p = ctx.enter_context(tc.tile_pool(name="dma", bufs=1))
    h1 = ctx.enter_context(tc.tile_pool(name="h1", bufs=1))
    h2 = ctx.enter_context(tc.tile_pool(name="h2", bufs=2))
    h2b = ctx.enter_context(tc.tile_pool(name="h2b", bufs=1))
    zab = ctx.enter_context(tc.tile_pool(name="zab", bufs=1))
    zcp = ctx.enter_context(tc.tile_pool(name="zcp", bufs=2))
    ep = ctx.enter_context(tc.tile_pool(name="ep", bufs=3))
    etp = ctx.enter_context(tc.tile_pool(name="etp", bufs=2))
    tlA = ctx.enter_context(tc.tile_pool(name="tlA", bufs=2))
    tl = ctx.enter_context(tc.tile_pool(name="tl", bufs=1))
    psum = ctx.enter_context(tc.tile_pool(name="psum", bufs=2, space="PSUM"))

    # one-time const tiles; their DMAs are issued later (inside h12(0))
    # so their descriptors don't clog the DMA engines ahead of the first
    # x chunks. Bias constants are columns of wqkb, registered in
    # const_aps as tile-slice APs so activation reads are dep-tracked on
    # the DMA (no barrier needed).
    wqkb = const.tile([P, 2 * npairs + 8], f32, tag="wqkb")
    for i, v in enumerate(_CONST_BIASES):
        nc.const_aps.aps[(f32, v)] = wqkb[:, 2 * npairs + i : 2 * npairs + i + 1]
    idpm = const.tile([P, 2 * P], f32r, tag="idpm")
    sacc = const.tile([P, 7], f32, tag="sacc")

    def const_loads():
        nc.sync.dma_start(wqkb[:], wqkb_d[:])
        nc.sync.dma_start(idpm[:], idpm_d[:].bitcast(f32r))
        # dummy [P,1] Exp: forces the act-table load to happen now, in the
        # DMA shadow, instead of right before the first real activation
        warm = const.tile([P, 1], f32, tag="warm")
        nc.scalar.activation(warm[:], wqkb[:, 0:1], Act.Exp)

    state = {}

    def h12(t, nchunk=1):
        """DMA + r^2 + 1/r, optionally sub-chunked to shorten the serial
        V->S->V warm-up chain of the very first pass."""
        x0t = dma_p.tile([P, F], bf16, tag="x0t")
        x1t = dma_p.tile([P, F], bf16, tag="x1t")
        wl0t = dma_p.tile([P, F], bf16, tag="wl0t")
        wl1t = dma_p.tile([P, F], bf16, tag="wl1t")
        fd = F // nchunk
        for k in range(nchunk):
            ck = bass.ts(t * nchunk + k, fd)
            cl = bass.ts(k, fd)
            nc.sync.dma_start(x0t[:, cl], x0_d[:, ck])
            nc.sync.dma_start(x1t[:, cl], x1_d[:, ck])
        sl = bass.ts(t, F)
        if t == 0:
            # const + wl descriptors go to the engines after the x chunks,
            # so the first compute's data is served first
            const_loads()
        nc.sync.dma_start(wl0t[:], wl0_d[:, sl])
        nc.sync.dma_start(wl1t[:], wl1_d[:, sl])

        sq0 = h1.tile([P, F], bf16, tag="sq0")
        sq1 = h1.tile([P, F], bf16, tag="sq1")
        rsq = h2.tile([P, F], bf16, tag="rsq")
        lr = h1.tile([P, F], f32, tag="lr")
        invr = h2.tile([P, F], bf16, tag="invr")
        fc = F // nchunk
        for k in range(nchunk):
            ck = bass.ts(k, fc)
            nc.vector.tensor_mul(sq0[:, ck], x0t[:, ck], x0t[:, ck])
            nc.vector.tensor_mul(sq1[:, ck], x1t[:, ck], x1t[:, ck])
            nc.vector.tensor_add(rsq[:, ck], sq0[:, ck], sq1[:, ck])
            nc.scalar.activation(lr[:, ck], rsq[:, ck], Act.Ln, bias=1e-30)
            nc.scalar.activation(invr[:, ck], lr[:, ck], Act.Exp, scale=-0.5)
        state[t] = dict(x0t=x0t, x1t=x1t, wl0t=wl0t, wl1t=wl1t, rsq=rsq,
                        invr=invr)

    def h3a(t):
        """Unit vector only -- the minimum V work before class dots."""
        st = state[t]
        y0 = h2b.tile([P, F], bf16, tag="y0")
        nc.vector.tensor_mul(y0[:], st["x0t"][:], st["invr"][:])
        y1 = h2b.tile([P, F], bf16, tag="y1")
        nc.vector.tensor_mul(y1[:], st["x1t"][:], st["invr"][:])
        es_ps = psum.tile([P, F], f32, tag="es")
        state[t].update(y0=y0, y1=y1, es=es_ps)

    def h3b_v(t):
        """Target dot (V). Emitted right after the class dots -- must
        precede h12(t+1) (its wl reads gate the next pass's wl DMAs)."""
        st = state[t]
        t1 = h1.tile([P, F], bf16, tag="t1")
        nc.vector.tensor_mul(t1[:], st["y0"][:], st["wl0t"][:])
        t2 = h1.tile([P, F], bf16, tag="t2")
        nc.vector.tensor_mul(t2[:], st["y1"][:], st["wl1t"][:])
        tt = h2b.tile([P, F], bf16, tag="tt")
        nc.vector.tensor_add(tt[:], t1[:], t2[:])
        state[t].update(tt=tt)

    def h3b_s(t):
        """e_t (closes the PSUM chain with stop=True) + r. No VectorE
        work, so it can splice anywhere after h3b_v(t)."""
        st = state[t]
        es_ps = st["es"]
        e_t = etp.tile([P, F], f32r, tag="et")
        nc.scalar.activation(e_t[:], st["tt"][:], Act.Exp, scale=S)
        for k in range(F // MM_N):
            ck = bass.ts(k, MM_N)
            nc.tensor.matmul(
                es_ps[:, ck], idpm[:, P:], e_t[:, ck], start=False, stop=True
            )
        # r on the otherwise-idle GpSimd engine (feeds only the soft sums)
        rs = h1.tile([P, F], bf16, tag="rs")
        nc.gpsimd.tensor_mul(rs[:], st["rsq"][:], st["invr"][:])
        state[t].update(rs=rs)

    def soft_accums(t):
        """Soft-loss partial sums; spliced late in the class loop so they
        don't block the dot/exp critical path. accum_out =
        reduce<op1=add>(op0 result): Sa = sum(min(r,1.5)),
        Sb = sum(max(r,2)); host: Sb - Sa - 0.5*N."""
        rs = state[t]["rs"]
        trash_a = h1.tile([P, F], bf16, tag="trash_a")
        nc.vector.tensor_scalar(
            trash_a[:], rs[:], 1.5, None, Alu.min, Alu.add,
            accum_out=sacc[:, 3 * t + 1 : 3 * t + 2],
        )
        trash_b = h1.tile([P, F], bf16, tag="trash_b")
        nc.vector.tensor_scalar(
            trash_b[:], rs[:], 2.0, None, Alu.max, Alu.add,
            accum_out=sacc[:, 3 * t + 2 : 3 * t + 3],
        )

    def tail_pre(t):
        """clip(t), 1-t^2 -- only needs tt; spliced into the class loop."""
        st = state[t]
        tcl = tlA.tile([P, F], bf16, tag="tcl")
        nc.vector.tensor_scalar(tcl[:], st["tt"][:], 1.0, -1.0, Alu.min, Alu.max)
        tsq = tl.tile([P, F], bf16, tag="tsq")
        nc.vector.tensor_mul(tsq[:], tcl[:], tcl[:])
        u = tlA.tile([P, F], bf16, tag="u")
        nc.vector.tensor_scalar(u[:], tsq[:], -1.0, 1.0, Alu.mult, Alu.add)
        state[t].update(tcl=tcl, u=u)

    def classes(t, spliced):
        """Class dot/exp/matmul loop for pass t; `spliced` maps a class
        index to closures emitted after that class (the previous pass's
        tail stages -- small V legs + S legs that fill ScalarE's queue)."""
        st = state[t]
        y0, y1, es_ps = st["y0"], st["y1"], st["es"]
        n_exp = 2 if sym else 1
        for c in range(npairs):
            za = zab.tile([P, F], bf16, tag="za")
            nc.vector.tensor_scalar(
                za[:], y0[:], wqkb[:, 2 * c : 2 * c + 1], None, Alu.mult
            )
            zb = zab.tile([P, F], bf16, tag="zb")
            nc.vector.tensor_scalar(
                zb[:], y1[:], wqkb[:, 2 * c + 1 : 2 * c + 2], None, Alu.mult
            )
            zc = zcp.tile([P, F], bf16, tag="zc")
            nc.vector.tensor_add(zc[:], za[:], zb[:])
            for j in range(n_exp):
                e = ep.tile([P, F], f32r, tag="e")
                nc.scalar.activation(e[:], zc[:], Act.Exp, scale=(S, -S)[j])
                start = (c == 0) and (j == 0)
                for k in range(F // MM_N):
                    ck = bass.ts(k, MM_N)
                    nc.tensor.matmul(
                        es_ps[:, ck], idpm[:, :P], e[:, ck], start=start, stop=False
                    )
            for fn in spliced.get(c, ()):
                fn()

    def tail_post_ops(t, parts):
        """Stage closures for pass t's tail over free-dim sub-ranges.
        parts: list of (offset, length, lden_col). Returns per-stage lists
        of closures (each stage covers all parts, so a later part's
        ScalarE leg hides an earlier part's VectorE leg and vice versa)."""
        st = state[t]
        # one allocation per tag per pass; parts write disjoint slices
        lnu = tl.tile([P, F], bf16, tag="lnu")
        sqru = tl.tile([P, F], bf16, tag="sqru")
        nump = tl.tile([P, F], bf16, tag="nump")
        e_nn = tl.tile([P, F], f32, tag="enn")
        w = tl.tile([P, F], f32, tag="w")
        trash = tl.tile([P, F], bf16, tag="trash_l")

        def mk(stage, p):
            off, ln, col = parts[p]
            fs = slice(off, off + ln)

            def s_lnu():
                nc.scalar.activation(lnu[:, fs], st["u"][:, fs], Act.Ln, bias=1e-12)
                nc.scalar.activation(
                    sqru[:, fs], lnu[:, fs], Act.Exp, scale=0.5, bias=LN_TAN_M
                )

            def v_nump():
                nc.vector.tensor_tensor(
                    nump[:, fs], st["tcl"][:, fs], sqru[:, fs], Alu.subtract
                )

            def s_enn():
                nc.scalar.activation(
                    e_nn[:, fs], nump[:, fs], Act.Exp, scale=-S * COS_M, bias=-LSH
                )

            def v_w():
                nc.vector.tensor_mul(w[:, fs], st["es"][:, fs], e_nn[:, fs])

            def s_lnacc():
                nc.scalar.activation(
                    trash[:, fs], w[:, fs], Act.Ln, bias=math.exp(-LSH),
                    accum_out=sacc[:, col : col + 1],
                )

            return [s_lnu, v_nump, s_enn, v_w, s_lnacc][stage]

        return [[mk(stage, p) for p in range(len(parts))] for stage in range(5)]

    # ---- software-pipelined emission ----
    # Rule: while ScalarE is consuming a pass's dot stream (2 exps per
    # dot), VectorE must not detour -- all non-dot V work is emitted
    # after that pass's dots.
    h12(0, nchunk=2)
    h3a(0)
    classes(0, {})
    h3b_v(0)
    h12(1)
    h3a(1)
    # pass-0's e_t/tail work rides inside pass-1's class loop: pass-1 dots
    # reach ScalarE with minimal V detour in between
    spliced = {0: [lambda: h3b_s(0), lambda: tail_pre(0)]}
    stages = tail_post_ops(0, [(0, F, 0)])
    for i, fns in enumerate(stages):
        spliced.setdefault(min(1 + i, npairs - 1), []).extend(fns)
    classes(1, spliced)
    h3b_v(1)
    h3b_s(1)
    tail_pre(1)
    soft_accums(0)
    soft_accums(1)
    # final tail: two interleaved half-chunks to overlap the V<->S chain
    fstages = tail_post_ops(1, [(0, F // 2, 3), (F // 2, F // 2, 6)])
    for fns in fstages:
        for fn in fns:
            fn()

    nc.sync.dma_start(out_d[:], sacc[:])


_NC_CACHE = {}


def _get_graph(sym: bool):
    if sym not in _NC_CACHE:
        _NC_CACHE[sym] = _build_graph(sym)
    return _NC_CACHE[sym]


def kernel(x, labels, weight):
    import ml_dtypes

    BF = ml_dtypes.bfloat16
    x = np.asarray(x, dtype=np.float32)
    labels = np.asarray(labels).astype(np.int64)
    w = np.asarray(weight, dtype=np.float32)

    wb = w.astype(BF)
    sym = w.shape[0] == NCLS and np.array_equal(
        wb[NCLS // 2 :], -wb[: NCLS // 2]
    )
    npairs = 5 if sym else NCLS
    nc = _get_graph(sym)

    wqkb_row = np.zeros(2 * npairs + 8, dtype=np.float32)
    wqkb_row[: 2 * npairs] = wb[:npairs].astype(np.float32).reshape(-1)
    wqkb_row[2 * npairs : 2 * npairs + len(_CONST_BIASES)] = _CONST_BIASES
    wqkb = np.ascontiguousarray(np.tile(wqkb_row.reshape(1, -1), (P, 1)))
    idpm = np.ascontiguousarray(
        np.concatenate([np.eye(P, dtype=np.float32), -np.eye(P, dtype=np.float32)],
                       axis=1)
    )

    wl = wb[labels]                      # [N, 2] bf16 gather on host
    x0 = x[:, 0].astype(BF)
    x1 = x[:, 1].astype(BF)

    in_maps = []
    for i in range(N_CORES):
        sl = slice(i * NC_ROWS, (i + 1) * NC_ROWS)
        in_maps.append(
            {
                "x0": np.ascontiguousarray(x0[sl]).reshape(P, PF),
                "x1": np.ascontiguousarray(x1[sl]).reshape(P, PF),
                "wl0": np.ascontiguousarray(wl[sl, 0]).reshape(P, PF),
                "wl1": np.ascontiguousarray(wl[sl, 1]).reshape(P, PF),
                "wqkb": wqkb,
                "idpm": idpm,
            }
        )

    trace = os.environ.get("KTRACE", "0") == "1"
    res = run_bass_kernel_spmd(nc, in_maps, core_ids=list(range(N_CORES)), trace=trace)
    if getattr(res, "exec_time_ns", None):
        print(f"HW exec time: {res.exec_time_ns} ns")

    lden_sum = 0.0
    soft_sum = -0.5 * N
    for i in range(N_CORES):
        o = np.asarray(res.results[i]["out"], dtype=np.float64)
        lden_sum += o[:, 0].sum() + o[:, 3].sum() + o[:, 6].sum()
        soft_sum += (o[:, 2].sum() - o[:, 1].sum()) + (o[:, 5].sum() - o[:, 4].sum())

    loss = lden_sum / N + LSH + (LBDA / 2.0) * (soft_sum / N)
    return np.float32(loss)


if __name__ == "__main__":
    rng = np.random.default_rng(0)
    x = rng.standard_normal((N, 2), dtype=np.float32)
    labels = rng.integers(0, 10, size=(N,)).astype(np.int64)
    w = np.array(
        [[1, 0], [0.809, 0.588], [0.309, 0.951], [-0.309, 0.951], [-0.809, 0.588],
         [-1, 0], [-0.809, -0.588], [-0.309, -0.951], [0.309, -0.951], [0.809, -0.588]],
        dtype=np.float32,
    )
    print(kernel(x, labels, w))
